# revision 23
# baseline (speedup 1.0000x reference)
"""Bass/Trainium2 kernel for nn_Attention_42305427865835.

Computes, for d_hidden [B,N,D], encoder_outputs [B,Lin,E], W1 [E+N*D, D],
b1 [D], w2 [D]:
    dec_proj = d_flat @ W1[:N*D] + b1                    # [B, D]
    enc_proj = enc @ W1[N*D:]                            # [B, Lin, E->D]
    scores   = tanh(enc_proj + dec_proj[:,None,:]) @ w2  # [B, Lin]
    out      = softmax(scores, axis=-1)
sharded data-parallel over batch, 4 batches per core on 8 cores.

Device-side layout is transposed ("T layout": D/E on partitions, Lin on the
free axis) so the contraction over E maps onto the PE array and the
dec_proj/b1 bias-add rides the ScalarE activation's per-partition bias.

The enc matmul (the dominant FLOPs) runs in fp8e4 with
MatmulPerfMode.DoubleRow: host pre-scales enc by 32 and W1_e by 8192
(keeping both inside fp8e4's +-240 range), packs the contraction as
[P, etile, free] so an e-tile PAIR is one K=256 DoubleRow matmul, and the
tanh activation's scale=2^-18 undoes the scaling exactly.  The score matmul
stays bf16 (fp8 there would blow the error budget).  Simulated end-to-end
absmax-relative error 1.83e-2 (gate 2e-2); the same simulator matches the
bf16 baseline's hardware error to 3 digits.

Softmax: scores for the 4 Lin-chunks of a batch land on PSUM partitions
{0,32,64,96} of one bank (tile_position picks the column group), so ONE Exp
activation covers the whole batch and its accum_out gives per-chunk sums.
The bank is memset to -100 first so unused partitions exp to 0, making the
ones-vector partition-sum matmul exact; gpsimd.partition_broadcast spreads
1/sum back across partitions for the final scale.  The partition-sum matmul
reuses element [0,0] of the score bank (no spare PSUM bank exists), and the
tail is pipelined across two chunk slots so the PE never waits on the Exp.

Score matmuls are emitted one chunk behind the enc matmuls so the PE queue
never head-blocks on the tanh that produces their input.  W1_d comes in two
half-tensors (d columns 0:256 / 256:512) so the dec matmuls can start after
only half the weight bytes have landed.

Softmax skips the max-subtraction: |scores| <= ||w2||_1 ~ 11, well inside
exp's fp32 range.
"""

import numpy as np

B, LIN, E, D, N = 32, 2048, 512, 512, 2
NCORES = 8
BPC = B // NCORES      # batches per core
P = 128                # SBUF partitions
ETILES = E // P        # 4
DTILES = D // P        # 4
ND = N * D             # 1024
KTILES = ND // P       # 8
LCHW = 512             # Lin chunk width (one PSUM bank of fp32)
LCH = LIN // LCHW      # 4

ENC_SCALE = 32.0       # enc pre-scale into fp8e4
W1E_SCALE = 8192.0     # W1_e pre-scale into fp8e4
INV_SCALE = 1.0 / (ENC_SCALE * W1E_SCALE)   # 2^-18, exact

# wmisc (bf16): dec-hidden columns + w2 columns
DH_LEN = KTILES * BPC          # 32: [k, b] -> d_flat[b, k*P+p]
W2_OFF = DH_LEN
W2_LEN = DTILES                # 4:  [a]    -> w2[a*P+p]
WMISC = DH_LEN + W2_LEN        # 36
DHALF = D // 2                 # 256
W1E_LEN = ETILES * D           # 2048: [e, d] -> W1_e[e*P+p, d] (fp8)

SCP = 3 * 32 + 1               # 97: score rows live at partitions {0,32,64,96}

TRACE = False
TRACE_KWARGS = {}
LAST_RESULT = None

_CACHE = {}


def _build():
    import concourse.bacc as bacc
    import concourse.mybir as mybir
    import concourse.tile as tile
    from concourse.bass import ts

    from concourse import bass_isa

    f32 = mybir.dt.float32
    bf16 = mybir.dt.bfloat16
    fp8 = mybir.dt.float8e4
    AF = mybir.ActivationFunctionType
    DR = mybir.MatmulPerfMode.DoubleRow

    nc = bacc.Bacc("TRN2", target_bir_lowering=False)

    encC_h = nc.dram_tensor(
        "encC", [BPC, LCH, P, ETILES, LCHW], fp8, kind="ExternalInput"
    )
    head8_h = nc.dram_tensor("head8", [P, 2 * W1E_LEN], fp8, kind="ExternalInput")
    wmisc_h = nc.dram_tensor("wmisc", [P, WMISC], bf16, kind="ExternalInput")
    w1d0_h = nc.dram_tensor("w1d0", [P, KTILES, DHALF], bf16, kind="ExternalInput")
    w1d1_h = nc.dram_tensor("w1d1", [P, KTILES, DHALF], bf16, kind="ExternalInput")
    b1z_h = nc.dram_tensor("b1z", [P, DTILES], f32, kind="ExternalInput")
    out_h = nc.dram_tensor("out", [BPC, LIN], f32, kind="ExternalOutput")

    with tile.TileContext(nc) as tc:
        with (
            tc.tile_pool(name="persist", bufs=1) as wp,
            tc.tile_pool(name="encp", bufs=BPC * LCH) as encp,
            tc.tile_pool(name="attnp", bufs=20) as attnp,
            tc.tile_pool(name="smp", bufs=2) as smp,
            tc.tile_pool(name="mainps", bufs=3, space="PSUM") as mainps,
            tc.tile_pool(name="scpsp", bufs=1, space="PSUM") as scpsp,
            tc.tile_pool(name="decps", bufs=1, space="PSUM") as decps,
        ):
            # --- critical path: w1e + first enc chunk fused in ONE DMA ---
            head_sb = wp.tile([P, 2 * ETILES, LCHW], fp8, tag="head8")
            nc.sync.dma_start(
                out=head_sb, in_=head8_h.rearrange("p (e d) -> p e d", e=2 * ETILES)
            )
            w1e_sb = head_sb[:, 0:ETILES, :]

            enc_tiles = [
                [
                    head_sb[:, ETILES : 2 * ETILES, :]
                    if (b == 0 and lc == 0)
                    else encp.tile(
                        [P, ETILES, LCHW], fp8, tag="enc", name=f"enc_b{b}l{lc}"
                    )
                    for lc in range(LCH)
                ]
                for b in range(BPC)
            ]

            wmisc_sb = wp.tile([P, WMISC], bf16, tag="wmisc")
            nc.sync.dma_start(out=wmisc_sb, in_=wmisc_h[:, :])
            w1d_sb = [
                wp.tile([P, KTILES, DHALF], bf16, tag=f"w1d{h}", name=f"w1d{h}")
                for h in range(2)
            ]
            nc.sync.dma_start(out=w1d_sb[0], in_=w1d0_h[:, :, :])
            b1_sb = wp.tile([P, DTILES], f32, tag="b1z")
            nc.sync.dma_start(out=b1_sb, in_=b1z_h[:, :])
            nc.sync.dma_start(out=w1d_sb[1], in_=w1d1_h[:, :, :])

            dh_sb = wmisc_sb[:, 0:DH_LEN].rearrange("p (k b) -> p k b", k=KTILES)
            w2_sb = wmisc_sb[:, W2_OFF : W2_OFF + W2_LEN]

            decb = wp.tile([P, DTILES, BPC], f32, tag="decb")

            def emit_dec(js):
                # dec_projT + b1 bias columns: [p, dtile, batch]
                for j in js:
                    dps = decps.tile([P, BPC], f32, tag="d", name=f"decps{j}")
                    for k in range(KTILES):
                        nc.tensor.matmul(
                            out=dps,
                            lhsT=w1d_sb[j // 2][:, k, ts(j % 2, P)],
                            rhs=dh_sb[:, k, :],
                            start=(k == 0),
                            stop=(k == KTILES - 1),
                        )
                    nc.vector.tensor_scalar_add(
                        out=decb[:, j, :], in0=dps, scalar1=b1_sb[:, j : j + 1]
                    )

            # remaining enc chunks, in consumption order (Sync trigger pacing
            # naturally prioritizes earlier chunks)
            for b in range(BPC):
                for lc in range(LCH):
                    if b == 0 and lc == 0:
                        continue
                    nc.sync.dma_start(out=enc_tiles[b][lc], in_=encC_h[b, lc])

            # --- main loop over 2-chunk slots ---
            # Each slot computes TWO Lin-chunks: the four j-groups land in
            # [P, 2, LCHW] double-bank PSUM tiles (ring of 3) so ONE tanh
            # activation covers both chunks of a j (same per-partition
            # dec-bias), halving the ACT per-op overhead count.
            # Scores for batch b are emitted after batch b+1's first slot
            # as column-tiled quads: the 4 chunks' M=1 matmuls target
            # distinct 32-column groups (partitions 0/32/64/96), so the PE
            # array runs each quad's 4 streams concurrently.
            slots = [(b, h) for b in range(BPC) for h in range(LCH // 2)]
            scs_tiles = {}
            attn_tiles = {}
            sume_tiles = {}

            def emit_scores_batch(b):
                sc = scs_tiles[b]
                for j in range(DTILES):
                    for lc in range(LCH):
                        at = attn_tiles[(b, lc // 2)][j]
                        nc.tensor.matmul(
                            out=sc[32 * lc : 32 * lc + 1, :],
                            lhsT=w2_sb[:, j : j + 1],
                            rhs=at[:, lc % 2, :],
                            start=(j == 0),
                            stop=(j == DTILES - 1),
                            tile_position=(0, 32 * lc),
                        )
                for h in range(LCH // 2):
                    attn_tiles.pop((b, h))

            def emit_exp(b):
                # one Exp for all 4 chunks (rows 0/32/64/96 + zeroed filler)
                erow = smp.tile([SCP, LCHW], f32, tag="erow", name=f"erow{b}")
                sume = smp.tile([SCP, 1], f32, tag="sume", name=f"sume{b}")
                nc.scalar.activation(
                    out=erow, in_=scs_tiles[b], func=AF.Exp, bias=0.0, scale=1.0,
                    accum_out=sume,
                )
                sume_tiles[b] = (erow, sume)

            def emit_tail2(b):
                # all-partition sum of per-chunk exp sums -> 1/sum -> scale
                erow, sume = sume_tiles.pop(b)
                scs_tiles.pop(b)
                sumall = smp.tile([SCP, 1], f32, tag="sumall", name=f"sumall{b}")
                nc.gpsimd.partition_all_reduce(
                    sumall, sume, SCP, bass_isa.ReduceOp.add
                )
                rinv97 = smp.tile([SCP, 1], f32, tag="rinv97", name=f"rinv97{b}")
                nc.vector.reciprocal(out=rinv97, in_=sumall)
                orow = smp.tile([SCP, LCHW], f32, tag="orow", name=f"orow{b}")
                nc.vector.tensor_scalar_mul(out=orow, in0=erow, scalar1=rinv97)
                nc.sync.dma_start(
                    out=out_h[b : b + 1, :].rearrange("o (c w) -> o c w", c=LCH),
                    in_=orow[0 : 3 * 32 + 1 : 32, :],
                )

            for i, (b, h) in enumerate(slots):
                ca, cb = 2 * h, 2 * h + 1
                mpss = []
                for j in range(DTILES):
                    mps = mainps.tile(
                        [P, 2, LCHW], f32, tag="m", name=f"mps_b{b}h{h}j{j}"
                    )
                    for c in (0, 1):
                        for t in range(ETILES // 2):
                            nc.tensor.matmul(
                                out=mps[:, c, :],
                                lhsT=w1e_sb[:, 2 * t : 2 * t + 2, ts(j, P)],
                                rhs=enc_tiles[b][ca + c][:, 2 * t : 2 * t + 2, :],
                                start=(t == 0),
                                stop=(t == ETILES // 2 - 1),
                                perf_mode=DR,
                            )
                    mpss.append(mps)

                if i == 0:
                    emit_dec((0, 1, 2, 3))
                if h == 0 and b >= 2:
                    emit_tail2(b - 2)
                if h == 1 and b >= 1:
                    emit_scores_batch(b - 1)

                attns = []
                for j in range(DTILES):
                    at = attnp.tile(
                        [P, 2, LCHW], bf16, tag="attn", name=f"attn_b{b}h{h}j{j}"
                    )
                    nc.scalar.activation(
                        out=at,
                        in_=mpss[j],
                        func=AF.Tanh,
                        bias=decb[:, j, b : b + 1],
                        scale=INV_SCALE,
                    )
                    attns.append(at)
                attn_tiles[(b, h)] = attns
                if h == 1:
                    if b >= 1:
                        # emitted after this slot's tanhs so the in-order ACT
                        # queue never parks on the Exp while tanh is ready
                        emit_exp(b - 1)
                    # score bank for batch b (gen b): after exp(b-1), its
                    # only reader of gen b-1, and before scores(b) quads
                    sc = scpsp.tile([SCP, LCHW], f32, tag="sc", name=f"sc{b}")
                    scs_tiles[b] = sc
                    nc.vector.memset(sc, -100.0)

            b_last = BPC - 1
            emit_tail2(b_last - 1)
            emit_scores_batch(b_last)
            emit_exp(b_last)
            emit_tail2(b_last)
    nc.compile()
    return nc


def _prep_in_maps(d_hidden, encoder_outputs, W1, b1, w2):
    import ml_dtypes

    bf = ml_dtypes.bfloat16
    f8 = ml_dtypes.float8_e4m3
    d_hidden = np.ascontiguousarray(np.asarray(d_hidden), dtype=np.float32)
    encoder_outputs = np.asarray(encoder_outputs)
    W1 = np.ascontiguousarray(np.asarray(W1), dtype=np.float32)
    b1 = np.ascontiguousarray(np.asarray(b1), dtype=np.float32)
    w2 = np.ascontiguousarray(np.asarray(w2), dtype=np.float32)

    W1d, W1e = W1[:ND], W1[ND:]
    w1e8 = np.ascontiguousarray(
        (W1e * W1E_SCALE)
        .reshape(ETILES, P, D)
        .transpose(1, 0, 2)
        .reshape(P, W1E_LEN)
        .astype(f8)
    )
    w1dk = W1d.reshape(KTILES, P, D).transpose(1, 0, 2).astype(bf)  # [P, k, D]
    w1d0 = np.ascontiguousarray(w1dk[:, :, :DHALF])
    w1d1 = np.ascontiguousarray(w1dk[:, :, DHALF:])
    b1z = np.ascontiguousarray(b1.reshape(DTILES, P).T)

    in_maps = []
    for c in range(NCORES):
        bs = slice(c * BPC, (c + 1) * BPC)
        encT = (
            np.asarray(encoder_outputs[bs], dtype=np.float32).transpose(0, 2, 1)
            * ENC_SCALE
        )  # [BPC, E, LIN] scaled
        encC = np.ascontiguousarray(
            encT.reshape(BPC, ETILES, P, LCH, LCHW)
            .transpose(0, 3, 2, 1, 4)
            .astype(f8)
        )
        dhT = np.ascontiguousarray(d_hidden[bs].reshape(BPC, ND).T)  # [ND, BPC]
        wmisc = np.zeros((P, WMISC), dtype=bf)
        wmisc[:, 0:DH_LEN] = (
            dhT.reshape(KTILES, P, BPC).transpose(1, 0, 2).reshape(P, DH_LEN)
            .astype(bf)
        )
        wmisc[:, W2_OFF : W2_OFF + W2_LEN] = w2.reshape(DTILES, P).T.astype(bf)
        head8 = np.concatenate(
            [w1e8, encC[0, 0].reshape(P, ETILES * LCHW)], axis=1
        )
        in_maps.append(
            {
                "encC": encC,
                "head8": head8,
                "wmisc": wmisc,
                "w1d0": w1d0,
                "w1d1": w1d1,
                "b1z": b1z,
            }
        )
    return in_maps


def kernel(d_hidden, encoder_outputs, W1, b1, w2):
    global LAST_RESULT
    from concourse import bass_utils

    if "nc" not in _CACHE:
        _CACHE["nc"] = _build()
    nc = _CACHE["nc"]

    in_maps = _prep_in_maps(d_hidden, encoder_outputs, W1, b1, w2)
    res = bass_utils.run_bass_kernel_spmd(
        nc,
        in_maps,
        core_ids=list(range(NCORES)),
        trace=TRACE,
        **TRACE_KWARGS,
    )
    LAST_RESULT = res
    return np.concatenate([r["out"] for r in res.results], axis=0)


# revision 25
# speedup vs baseline: 1.1885x; 1.1885x over previous
"""Bass/Trainium2 kernel for nn_Attention_42305427865835.

Computes, for d_hidden [B,N,D], encoder_outputs [B,Lin,E], W1 [E+N*D, D],
b1 [D], w2 [D]:
    dec_proj = d_flat @ W1[:N*D] + b1                    # [B, D]
    enc_proj = enc @ W1[N*D:]                            # [B, Lin, E->D]
    scores   = tanh(enc_proj + dec_proj[:,None,:]) @ w2  # [B, Lin]
    out      = softmax(scores, axis=-1)
sharded data-parallel over batch, 4 batches per core on 8 cores.

Device-side layout is transposed ("T layout": D/E on partitions, Lin on the
free axis) so the contraction over E maps onto the PE array and the
dec_proj/b1 bias-add rides the ScalarE activation's per-partition bias.

The enc matmul (the dominant FLOPs) runs in fp8e4 with
MatmulPerfMode.DoubleRow: host pre-scales enc by 32 and W1_e by 8192
(keeping both inside fp8e4's +-240 range), packs the contraction as
[P, etile, free] so an e-tile PAIR is one K=256 DoubleRow matmul, and the
tanh activation's scale=2^-18 undoes the scaling exactly.  The score matmul
stays bf16 (fp8 there would blow the error budget).  Simulated end-to-end
absmax-relative error 1.83e-2 (gate 2e-2); the same simulator matches the
bf16 baseline's hardware error to 3 digits.

Softmax: scores for the 4 Lin-chunks of a batch land on PSUM partitions
{0,32,64,96} of one bank (tile_position picks the column group), so ONE Exp
activation covers the whole batch and its accum_out gives per-chunk sums.
The bank is memset to -100 first so unused partitions exp to 0, making the
ones-vector partition-sum matmul exact; gpsimd.partition_broadcast spreads
1/sum back across partitions for the final scale.  The partition-sum matmul
reuses element [0,0] of the score bank (no spare PSUM bank exists), and the
tail is pipelined across two chunk slots so the PE never waits on the Exp.

Score matmuls are emitted one chunk behind the enc matmuls so the PE queue
never head-blocks on the tanh that produces their input.  W1_d comes in two
half-tensors (d columns 0:256 / 256:512) so the dec matmuls can start after
only half the weight bytes have landed.

Softmax skips the max-subtraction: |scores| <= ||w2||_1 ~ 11, well inside
exp's fp32 range.
"""

import numpy as np

B, LIN, E, D, N = 32, 2048, 512, 512, 2
NCORES = 8
BPC = B // NCORES      # batches per core
P = 128                # SBUF partitions
ETILES = E // P        # 4
DTILES = D // P        # 4
ND = N * D             # 1024
KTILES = ND // P       # 8
LCHW = 512             # Lin chunk width (one PSUM bank of fp32)
LCH = LIN // LCHW      # 4

ENC_SCALE = 32.0       # enc pre-scale into fp8e4
W1E_SCALE = 8192.0     # W1_e pre-scale into fp8e4
INV_SCALE = 1.0 / (ENC_SCALE * W1E_SCALE)   # 2^-18, exact

# wmisc (bf16): dec-hidden columns + w2 columns
DH_LEN = KTILES * BPC          # 32: [k, b] -> d_flat[b, k*P+p]
W2_OFF = DH_LEN
W2_LEN = DTILES                # 4:  [a]    -> w2[a*P+p]
WMISC = DH_LEN + W2_LEN        # 36
DHALF = D // 2                 # 256
W1E_LEN = ETILES * D           # 2048: [e, d] -> W1_e[e*P+p, d] (fp8)

SCP = 3 * 32 + 1               # 97: score rows live at partitions {0,32,64,96}

TRACE = False
TRACE_KWARGS = {}
LAST_RESULT = None

_CACHE = {}


def _build():
    import concourse.bacc as bacc
    import concourse.mybir as mybir
    import concourse.tile as tile
    from concourse.bass import ts

    from concourse import bass_isa

    f32 = mybir.dt.float32
    bf16 = mybir.dt.bfloat16
    fp8 = mybir.dt.float8e4
    AF = mybir.ActivationFunctionType
    DR = mybir.MatmulPerfMode.DoubleRow

    nc = bacc.Bacc("TRN2", target_bir_lowering=False)

    encC_h = nc.dram_tensor(
        "encC", [BPC, LCH, P, ETILES, LCHW], fp8, kind="ExternalInput"
    )
    head8_h = nc.dram_tensor("head8", [P, 2 * W1E_LEN], fp8, kind="ExternalInput")
    wmisc_h = nc.dram_tensor("wmisc", [P, WMISC], bf16, kind="ExternalInput")
    w1d0_h = nc.dram_tensor("w1d0", [P, KTILES, DHALF], bf16, kind="ExternalInput")
    w1d1_h = nc.dram_tensor("w1d1", [P, KTILES, DHALF], bf16, kind="ExternalInput")
    b1z_h = nc.dram_tensor("b1z", [P, DTILES], f32, kind="ExternalInput")
    out_h = nc.dram_tensor("out", [BPC, LIN], f32, kind="ExternalOutput")

    with tile.TileContext(nc) as tc:
        with (
            tc.tile_pool(name="persist", bufs=1) as wp,
            tc.tile_pool(name="encp", bufs=BPC * LCH) as encp,
            tc.tile_pool(name="attnp", bufs=20) as attnp,
            tc.tile_pool(name="smp", bufs=2) as smp,
            tc.tile_pool(name="mainps", bufs=3, space="PSUM") as mainps,
            tc.tile_pool(name="scpsp", bufs=1, space="PSUM") as scpsp,
            tc.tile_pool(name="decps", bufs=1, space="PSUM") as decps,
        ):
            # --- critical path: w1e + first enc chunk fused in ONE DMA ---
            head_sb = wp.tile([P, 2 * ETILES, LCHW], fp8, tag="head8")
            nc.sync.dma_start(
                out=head_sb, in_=head8_h.rearrange("p (e d) -> p e d", e=2 * ETILES)
            )
            w1e_sb = head_sb[:, 0:ETILES, :]

            enc_tiles = [
                [
                    head_sb[:, ETILES : 2 * ETILES, :]
                    if (b == 0 and lc == 0)
                    else encp.tile(
                        [P, ETILES, LCHW], fp8, tag="enc", name=f"enc_b{b}l{lc}"
                    )
                    for lc in range(LCH)
                ]
                for b in range(BPC)
            ]

            w1d_sb = [
                wp.tile([P, KTILES, DHALF], bf16, tag=f"w1d{h}", name=f"w1d{h}")
                for h in range(2)
            ]
            nc.sync.dma_start(out=w1d_sb[0], in_=w1d0_h[:, :, :])
            wmisc_sb = wp.tile([P, WMISC], bf16, tag="wmisc")
            nc.sync.dma_start(out=wmisc_sb, in_=wmisc_h[:, :])
            b1_sb = wp.tile([P, DTILES], f32, tag="b1z")
            nc.sync.dma_start(out=b1_sb, in_=b1z_h[:, :])
            nc.sync.dma_start(out=w1d_sb[1], in_=w1d1_h[:, :, :])

            dh_sb = wmisc_sb[:, 0:DH_LEN].rearrange("p (k b) -> p k b", k=KTILES)
            w2_sb = wmisc_sb[:, W2_OFF : W2_OFF + W2_LEN]

            decb = wp.tile([P, DTILES, BPC], f32, tag="decb")

            def emit_dec(js):
                # dec_projT + b1 bias columns: [p, dtile, batch]
                for j in js:
                    dps = decps.tile([P, BPC], f32, tag="d", name=f"decps{j}")
                    for k in range(KTILES):
                        nc.tensor.matmul(
                            out=dps,
                            lhsT=w1d_sb[j // 2][:, k, ts(j % 2, P)],
                            rhs=dh_sb[:, k, :],
                            start=(k == 0),
                            stop=(k == KTILES - 1),
                        )
                    nc.vector.tensor_scalar_add(
                        out=decb[:, j, :], in0=dps, scalar1=b1_sb[:, j : j + 1]
                    )

            # remaining enc chunks, in consumption order (Sync trigger pacing
            # naturally prioritizes earlier chunks)
            for b in range(BPC):
                for lc in range(LCH):
                    if b == 0 and lc == 0:
                        continue
                    nc.sync.dma_start(out=enc_tiles[b][lc], in_=encC_h[b, lc])

            # --- main loop over 2-chunk slots ---
            # Each slot computes TWO Lin-chunks: the four j-groups land in
            # [P, 2, LCHW] double-bank PSUM tiles (ring of 3) so ONE tanh
            # activation covers both chunks of a j (same per-partition
            # dec-bias), halving the ACT per-op overhead count.
            # Scores for batch b are emitted after batch b+1's first slot
            # as column-tiled quads: the 4 chunks' M=1 matmuls target
            # distinct 32-column groups (partitions 0/32/64/96), so the PE
            # array runs each quad's 4 streams concurrently.
            slots = [(b, h) for b in range(BPC) for h in range(LCH // 2)]
            scs_tiles = {}
            attn_tiles = {}
            sume_tiles = {}

            def emit_scores_batch(b):
                sc = scs_tiles[b]
                for j in range(DTILES):
                    for lc in range(LCH):
                        at = attn_tiles[(b, lc // 2)][j]
                        nc.tensor.matmul(
                            out=sc[32 * lc : 32 * lc + 1, :],
                            lhsT=w2_sb[:, j : j + 1],
                            rhs=at[:, lc % 2, :],
                            start=(j == 0),
                            stop=(j == DTILES - 1),
                            tile_position=(0, 32 * lc),
                        )
                for h in range(LCH // 2):
                    attn_tiles.pop((b, h))

            def emit_exp(b):
                # one Exp for all 4 chunks (rows 0/32/64/96 + zeroed filler)
                erow = smp.tile([SCP, LCHW], f32, tag="erow", name=f"erow{b}")
                sume = smp.tile([SCP, 1], f32, tag="sume", name=f"sume{b}")
                nc.scalar.activation(
                    out=erow, in_=scs_tiles[b], func=AF.Exp, bias=0.0, scale=1.0,
                    accum_out=sume,
                )
                sume_tiles[b] = (erow, sume)

            def emit_tail2(b):
                # all-partition sum of per-chunk exp sums -> 1/sum -> scale
                erow, sume = sume_tiles.pop(b)
                scs_tiles.pop(b)
                sumall = smp.tile([SCP, 1], f32, tag="sumall", name=f"sumall{b}")
                nc.gpsimd.partition_all_reduce(
                    sumall, sume, SCP, bass_isa.ReduceOp.add
                )
                rinv97 = smp.tile([SCP, 1], f32, tag="rinv97", name=f"rinv97{b}")
                nc.vector.reciprocal(out=rinv97, in_=sumall)
                orow = smp.tile([SCP, LCHW], f32, tag="orow", name=f"orow{b}")
                nc.vector.tensor_scalar_mul(out=orow, in0=erow, scalar1=rinv97)
                nc.sync.dma_start(
                    out=out_h[b : b + 1, :].rearrange("o (c w) -> o c w", c=LCH),
                    in_=orow[0 : 3 * 32 + 1 : 32, :],
                )

            for i, (b, h) in enumerate(slots):
                ca, cb = 2 * h, 2 * h + 1
                mpss = []
                for j in range(DTILES):
                    mps = mainps.tile(
                        [P, 2, LCHW], f32, tag="m", name=f"mps_b{b}h{h}j{j}"
                    )
                    for c in (0, 1):
                        for t in range(ETILES // 2):
                            nc.tensor.matmul(
                                out=mps[:, c, :],
                                lhsT=w1e_sb[:, 2 * t : 2 * t + 2, ts(j, P)],
                                rhs=enc_tiles[b][ca + c][:, 2 * t : 2 * t + 2, :],
                                start=(t == 0),
                                stop=(t == ETILES // 2 - 1),
                                perf_mode=DR,
                            )
                    mpss.append(mps)

                if i == 0:
                    emit_dec((0, 1, 2, 3))
                if h == 0 and b >= 1:
                    emit_scores_batch(b - 1)
                if h == 1:
                    if b >= 1:
                        emit_tail2(b - 1)
                    # score bank for batch b; its only gen-(b-1) reader is
                    # exp(b-1), one slot back — must precede scores(b) quads
                    sc = scpsp.tile([SCP, LCHW], f32, tag="sc", name=f"sc{b}")
                    scs_tiles[b] = sc
                    nc.vector.memset(sc, -100.0)

                attns = []
                for j in range(DTILES):
                    at = attnp.tile(
                        [P, 2, LCHW], bf16, tag="attn", name=f"attn_b{b}h{h}j{j}"
                    )
                    nc.scalar.activation(
                        out=at,
                        in_=mpss[j],
                        func=AF.Tanh,
                        bias=decb[:, j, b : b + 1],
                        scale=INV_SCALE,
                    )
                    attns.append(at)
                attn_tiles[(b, h)] = attns
                if h == 0 and b >= 1:
                    # emitted after this slot's tanhs so the in-order ACT
                    # queue never parks on the Exp while tanh work is ready
                    emit_exp(b - 1)

            b_last = BPC - 1
            emit_scores_batch(b_last)
            emit_exp(b_last)
            emit_tail2(b_last)
    nc.compile()
    return nc


def _prep_in_maps(d_hidden, encoder_outputs, W1, b1, w2):
    import ml_dtypes

    bf = ml_dtypes.bfloat16
    f8 = ml_dtypes.float8_e4m3
    d_hidden = np.ascontiguousarray(np.asarray(d_hidden), dtype=np.float32)
    encoder_outputs = np.asarray(encoder_outputs)
    W1 = np.ascontiguousarray(np.asarray(W1), dtype=np.float32)
    b1 = np.ascontiguousarray(np.asarray(b1), dtype=np.float32)
    w2 = np.ascontiguousarray(np.asarray(w2), dtype=np.float32)

    W1d, W1e = W1[:ND], W1[ND:]
    w1e8 = np.ascontiguousarray(
        (W1e * W1E_SCALE)
        .reshape(ETILES, P, D)
        .transpose(1, 0, 2)
        .reshape(P, W1E_LEN)
        .astype(f8)
    )
    w1dk = W1d.reshape(KTILES, P, D).transpose(1, 0, 2).astype(bf)  # [P, k, D]
    w1d0 = np.ascontiguousarray(w1dk[:, :, :DHALF])
    w1d1 = np.ascontiguousarray(w1dk[:, :, DHALF:])
    b1z = np.ascontiguousarray(b1.reshape(DTILES, P).T)

    in_maps = []
    for c in range(NCORES):
        bs = slice(c * BPC, (c + 1) * BPC)
        encT = (
            np.asarray(encoder_outputs[bs], dtype=np.float32).transpose(0, 2, 1)
            * ENC_SCALE
        )  # [BPC, E, LIN] scaled
        encC = np.ascontiguousarray(
            encT.reshape(BPC, ETILES, P, LCH, LCHW)
            .transpose(0, 3, 2, 1, 4)
            .astype(f8)
        )
        dhT = np.ascontiguousarray(d_hidden[bs].reshape(BPC, ND).T)  # [ND, BPC]
        wmisc = np.zeros((P, WMISC), dtype=bf)
        wmisc[:, 0:DH_LEN] = (
            dhT.reshape(KTILES, P, BPC).transpose(1, 0, 2).reshape(P, DH_LEN)
            .astype(bf)
        )
        wmisc[:, W2_OFF : W2_OFF + W2_LEN] = w2.reshape(DTILES, P).T.astype(bf)
        head8 = np.concatenate(
            [w1e8, encC[0, 0].reshape(P, ETILES * LCHW)], axis=1
        )
        in_maps.append(
            {
                "encC": encC,
                "head8": head8,
                "wmisc": wmisc,
                "w1d0": w1d0,
                "w1d1": w1d1,
                "b1z": b1z,
            }
        )
    return in_maps


def kernel(d_hidden, encoder_outputs, W1, b1, w2):
    global LAST_RESULT
    from concourse import bass_utils

    if "nc" not in _CACHE:
        _CACHE["nc"] = _build()
    nc = _CACHE["nc"]

    in_maps = _prep_in_maps(d_hidden, encoder_outputs, W1, b1, w2)
    res = bass_utils.run_bass_kernel_spmd(
        nc,
        in_maps,
        core_ids=list(range(NCORES)),
        trace=TRACE,
        **TRACE_KWARGS,
    )
    LAST_RESULT = res
    return np.concatenate([r["out"] for r in res.results], axis=0)


# revision 37
# speedup vs baseline: 1.2254x; 1.0310x over previous
"""Bass/Trainium2 kernel for nn_Attention_42305427865835.

Computes, for d_hidden [B,N,D], encoder_outputs [B,Lin,E], W1 [E+N*D, D],
b1 [D], w2 [D]:
    dec_proj = d_flat @ W1[:N*D] + b1                    # [B, D]
    enc_proj = enc @ W1[N*D:]                            # [B, Lin, E->D]
    scores   = tanh(enc_proj + dec_proj[:,None,:]) @ w2  # [B, Lin]
    out      = softmax(scores, axis=-1)
sharded data-parallel over batch, 4 batches per core on 8 cores.

Device-side layout is transposed ("T layout": D/E on partitions, Lin on the
free axis) so the contraction over E maps onto the PE array and the
dec_proj/b1 bias-add rides the ScalarE activation's per-partition bias.

The enc matmul (the dominant FLOPs) runs in fp8e4 with
MatmulPerfMode.DoubleRow: host pre-scales enc by 32 and W1_e by 8192
(keeping both inside fp8e4's +-240 range), packs the contraction as
[P, etile, free] so an e-tile PAIR is one K=256 DoubleRow matmul, and the
tanh activation's scale=2^-18 undoes the scaling exactly.  The score matmul
stays bf16 (fp8 there would blow the error budget).  Simulated end-to-end
absmax-relative error 1.83e-2 (gate 2e-2); the same simulator matches the
bf16 baseline's hardware error to 3 digits.

Softmax: scores for the 4 Lin-chunks of a batch land on PSUM partitions
{0,32,64,96} of one bank (tile_position picks the column group), so ONE Exp
activation covers the whole batch and its accum_out gives per-chunk sums.
The bank is memset to -100 first so unused partitions exp to 0, making the
ones-vector partition-sum matmul exact; gpsimd.partition_broadcast spreads
1/sum back across partitions for the final scale.  The partition-sum matmul
reuses element [0,0] of the score bank (no spare PSUM bank exists), and the
tail is pipelined across two chunk slots so the PE never waits on the Exp.

Score matmuls are emitted one chunk behind the enc matmuls so the PE queue
never head-blocks on the tanh that produces their input.  W1_d comes in two
half-tensors (d columns 0:256 / 256:512) so the dec matmuls can start after
only half the weight bytes have landed.

Softmax skips the max-subtraction: |scores| <= ||w2||_1 ~ 11, well inside
exp's fp32 range.
"""

import numpy as np

B, LIN, E, D, N = 32, 2048, 512, 512, 2
NCORES = 8
BPC = B // NCORES      # batches per core
P = 128                # SBUF partitions
ETILES = E // P        # 4
DTILES = D // P        # 4
ND = N * D             # 1024
KTILES = ND // P       # 8
LCHW = 512             # Lin chunk width (one PSUM bank of fp32)
LCH = LIN // LCHW      # 4

ENC_SCALE = 32.0       # enc pre-scale into fp8e4
W1E_SCALE = 8192.0     # W1_e pre-scale into fp8e4
INV_SCALE = 1.0 / (ENC_SCALE * W1E_SCALE)   # 2^-18, exact

# wmisc (bf16): dec-hidden columns + w2 columns
DH_LEN = KTILES * BPC          # 32: [k, b] -> d_flat[b, k*P+p]
W2_OFF = DH_LEN
W2_LEN = DTILES                # 4:  [a]    -> w2[a*P+p]
WMISC = DH_LEN + W2_LEN        # 36
DHALF = D // 2                 # 256
W1E_LEN = ETILES * D           # 2048: [e, d] -> W1_e[e*P+p, d] (fp8)

SCP = 3 * 32 + 1               # 97: score rows live at partitions {0,32,64,96}

TRACE = False
TRACE_KWARGS = {}
LAST_RESULT = None

_CACHE = {}


def _build():
    import concourse.bacc as bacc
    import concourse.mybir as mybir
    import concourse.tile as tile
    from concourse.bass import ts

    from concourse import bass_isa

    f32 = mybir.dt.float32
    bf16 = mybir.dt.bfloat16
    fp8 = mybir.dt.float8e4
    AF = mybir.ActivationFunctionType
    DR = mybir.MatmulPerfMode.DoubleRow

    nc = bacc.Bacc("TRN2", target_bir_lowering=False)

    encC_h = nc.dram_tensor(
        "encC", [BPC, LCH, P, ETILES, LCHW], fp8, kind="ExternalInput"
    )
    head8_h = nc.dram_tensor("head8", [P, 2 * W1E_LEN], fp8, kind="ExternalInput")
    wmisc_h = nc.dram_tensor("wmisc", [P, WMISC], bf16, kind="ExternalInput")
    # w1d0b: first half of W1_d (d 0:256) ++ b1 columns, all bf16
    w1d0b_h = nc.dram_tensor(
        "w1d0b", [P, KTILES * DHALF + DTILES], bf16, kind="ExternalInput"
    )
    w1d1_h = nc.dram_tensor("w1d1", [P, KTILES, DHALF], bf16, kind="ExternalInput")
    out_h = nc.dram_tensor("out", [BPC, LIN], f32, kind="ExternalOutput")

    with tile.TileContext(nc) as tc:
        with (
            tc.tile_pool(name="persist", bufs=1) as wp,
            tc.tile_pool(name="encp", bufs=BPC - 1) as encp,
            tc.tile_pool(name="attnp", bufs=20) as attnp,
            tc.tile_pool(name="smp", bufs=2) as smp,
            tc.tile_pool(name="mainps", bufs=3, space="PSUM") as mainps,
            tc.tile_pool(name="scpsp", bufs=1, space="PSUM") as scpsp,
            tc.tile_pool(name="decps", bufs=1, space="PSUM") as decps,
        ):
            # --- critical path: w1e + first enc chunk fused in ONE DMA ---
            head_sb = wp.tile([P, 2 * ETILES, LCHW], fp8, tag="head8")
            nc.sync.dma_start(
                out=head_sb, in_=head8_h.rearrange("p (e d) -> p e d", e=2 * ETILES)
            )
            w1e_sb = head_sb[:, 0:ETILES, :]

            # batch 0: chunk-granular tiles (fast first-compute); 1-3: one
            # batch tile + one DMA each
            enc_b0 = [
                encp.tile([P, ETILES, LCHW], fp8, tag="enc0", name=f"enc0l{lc}")
                for lc in range(1, LCH)
            ]
            enc_bt = [
                encp.tile(
                    [P, LCH, ETILES, LCHW], fp8, tag="encb", name=f"encb{b}"
                )
                for b in range(1, BPC)
            ]
            enc_tiles = [[head_sb[:, ETILES : 2 * ETILES, :]] + enc_b0] + [
                [enc_bt[b - 1][:, lc] for lc in range(LCH)] for b in range(1, BPC)
            ]

            w1d0b_sb = wp.tile([P, KTILES * DHALF + DTILES], bf16, tag="w1d0b")
            nc.sync.dma_start(out=w1d0b_sb, in_=w1d0b_h[:, :])
            w1d_sb = [
                w1d0b_sb[:, 0 : KTILES * DHALF].rearrange(
                    "p (k d) -> p k d", k=KTILES
                ),
                wp.tile([P, KTILES, DHALF], bf16, tag="w1d1", name="w1d1"),
            ]
            b1_bf = w1d0b_sb[:, KTILES * DHALF :]
            b1_sb = wp.tile([P, DTILES], f32, tag="b1f")
            nc.scalar.copy(out=b1_sb, in_=b1_bf)
            wmisc_sb = wp.tile([P, WMISC], bf16, tag="wmisc")
            nc.sync.dma_start(out=wmisc_sb, in_=wmisc_h[:, :])
            nc.sync.dma_start(out=w1d_sb[1], in_=w1d1_h[:, :, :])

            dh_sb = wmisc_sb[:, 0:DH_LEN].rearrange("p (k b) -> p k b", k=KTILES)
            w2_sb = wmisc_sb[:, W2_OFF : W2_OFF + W2_LEN]

            decb = wp.tile([P, DTILES, BPC], f32, tag="decb")

            def emit_dec(js):
                # dec_projT + b1 bias columns: [p, dtile, batch]
                for j in js:
                    dps = decps.tile([P, BPC], f32, tag="d", name=f"decps{j}")
                    for k in range(KTILES):
                        nc.tensor.matmul(
                            out=dps,
                            lhsT=w1d_sb[j // 2][:, k, ts(j % 2, P)],
                            rhs=dh_sb[:, k, :],
                            start=(k == 0),
                            stop=(k == KTILES - 1),
                        )
                    nc.vector.tensor_scalar_add(
                        out=decb[:, j, :], in0=dps, scalar1=b1_sb[:, j : j + 1]
                    )

            # remaining enc DMAs, in consumption order (Sync trigger pacing
            # naturally prioritizes earlier data)
            for lc in range(1, LCH):
                nc.sync.dma_start(out=enc_b0[lc - 1], in_=encC_h[0, lc])
            for b in range(1, BPC):
                nc.sync.dma_start(
                    out=enc_bt[b - 1],
                    in_=encC_h[b].rearrange("l p e w -> p l e w"),
                )

            # --- main loop over 2-chunk slots ---
            # Each slot computes TWO Lin-chunks: the four j-groups land in
            # [P, 2, LCHW] double-bank PSUM tiles (ring of 3) so ONE tanh
            # activation covers both chunks of a j (same per-partition
            # dec-bias), halving the ACT per-op overhead count.
            # Scores for batch b are emitted after batch b+1's first slot
            # as column-tiled quads: the 4 chunks' M=1 matmuls target
            # distinct 32-column groups (partitions 0/32/64/96), so the PE
            # array runs each quad's 4 streams concurrently.
            slots = [(b, h) for b in range(BPC) for h in range(LCH // 2)]
            scs_tiles = {}
            attn_tiles = {}
            sume_tiles = {}

            def emit_scores_batch(b, js=tuple(range(DTILES))):
                sc = scs_tiles[b]
                for j in js:
                    for lc in range(LCH):
                        at = attn_tiles[(b, lc // 2)][j]
                        nc.tensor.matmul(
                            out=sc[32 * lc : 32 * lc + 1, :],
                            lhsT=w2_sb[:, j : j + 1],
                            rhs=at[:, lc % 2, :],
                            start=(j == 0),
                            stop=(j == DTILES - 1),
                            tile_position=(0, 32 * lc),
                        )
                if js[-1] == DTILES - 1:
                    for h in range(LCH // 2):
                        attn_tiles.pop((b, h))

            def emit_exp(b):
                # one Exp for all 4 chunks (rows 0/32/64/96 + zeroed filler)
                erow = smp.tile([SCP, LCHW], f32, tag="erow", name=f"erow{b}")
                sume = smp.tile([SCP, 1], f32, tag="sume", name=f"sume{b}")
                nc.scalar.activation(
                    out=erow, in_=scs_tiles[b], func=AF.Exp, bias=0.0, scale=1.0,
                    accum_out=sume,
                )
                sume_tiles[b] = (erow, sume)

            def emit_tail2(b):
                # all-partition sum of per-chunk exp sums -> 1/sum -> scale
                erow, sume = sume_tiles.pop(b)
                scs_tiles.pop(b)
                sumall = smp.tile([SCP, 1], f32, tag="sumall", name=f"sumall{b}")
                nc.gpsimd.partition_all_reduce(
                    sumall, sume, SCP, bass_isa.ReduceOp.add
                )
                rinv97 = smp.tile([SCP, 1], f32, tag="rinv97", name=f"rinv97{b}")
                nc.vector.reciprocal(out=rinv97, in_=sumall)
                orow = smp.tile([SCP, LCHW], f32, tag="orow", name=f"orow{b}")
                nc.vector.tensor_scalar_mul(out=orow, in0=erow, scalar1=rinv97)
                nc.sync.dma_start(
                    out=out_h[b : b + 1, :].rearrange("o (c w) -> o c w", c=LCH),
                    in_=orow[0 : 3 * 32 + 1 : 32, :],
                )

            for i, (b, h) in enumerate(slots):
                ca, cb = 2 * h, 2 * h + 1
                mpss = []
                for j in range(DTILES):
                    mps = mainps.tile(
                        [P, 2, LCHW], f32, tag="m", name=f"mps_b{b}h{h}j{j}"
                    )
                    for c in (0, 1):
                        for t in range(ETILES // 2):
                            nc.tensor.matmul(
                                out=mps[:, c, :],
                                lhsT=w1e_sb[:, 2 * t : 2 * t + 2, ts(j, P)],
                                rhs=enc_tiles[b][ca + c][:, 2 * t : 2 * t + 2, :],
                                start=(t == 0),
                                stop=(t == ETILES // 2 - 1),
                                perf_mode=DR,
                            )
                    mpss.append(mps)

                if i == 0:
                    emit_dec((0, 1, 2, 3))
                if h == 0 and b >= 1:
                    emit_scores_batch(b - 1)
                if h == 1:
                    if b >= 1:
                        emit_tail2(b - 1)
                    # score bank for batch b; its only gen-(b-1) reader is
                    # exp(b-1), one slot back — must precede scores(b) quads
                    sc = scpsp.tile([SCP, LCHW], f32, tag="sc", name=f"sc{b}")
                    scs_tiles[b] = sc
                    nc.vector.memset(sc, -100.0)

                attns = []
                for j in range(DTILES):
                    at = attnp.tile(
                        [P, 2, LCHW], bf16, tag="attn", name=f"attn_b{b}h{h}j{j}"
                    )
                    nc.scalar.activation(
                        out=at,
                        in_=mpss[j],
                        func=AF.Tanh,
                        bias=decb[:, j, b : b + 1],
                        scale=INV_SCALE,
                    )
                    attns.append(at)
                attn_tiles[(b, h)] = attns
                if h == 0 and b >= 1:
                    # emitted after this slot's tanhs so the in-order ACT
                    # queue never parks on the Exp while tanh work is ready
                    emit_exp(b - 1)
                if i == len(slots) - 1:
                    # last batch: j0/j1 quads run as soon as this slot's
                    # early tanhs land, shortening the tail
                    emit_scores_batch(b, (0, 1))

            b_last = BPC - 1
            emit_scores_batch(b_last, (2, 3))
            emit_exp(b_last)
            emit_tail2(b_last)
    nc.compile()
    return nc


def _prep_in_maps(d_hidden, encoder_outputs, W1, b1, w2):
    import ml_dtypes

    bf = ml_dtypes.bfloat16
    f8 = ml_dtypes.float8_e4m3
    d_hidden = np.ascontiguousarray(np.asarray(d_hidden), dtype=np.float32)
    encoder_outputs = np.asarray(encoder_outputs)
    W1 = np.ascontiguousarray(np.asarray(W1), dtype=np.float32)
    b1 = np.ascontiguousarray(np.asarray(b1), dtype=np.float32)
    w2 = np.ascontiguousarray(np.asarray(w2), dtype=np.float32)

    W1d, W1e = W1[:ND], W1[ND:]
    w1e8 = np.ascontiguousarray(
        (W1e * W1E_SCALE)
        .reshape(ETILES, P, D)
        .transpose(1, 0, 2)
        .reshape(P, W1E_LEN)
        .astype(f8)
    )
    w1dk = W1d.reshape(KTILES, P, D).transpose(1, 0, 2).astype(bf)  # [P, k, D]
    w1d0b = np.concatenate(
        [
            w1dk[:, :, :DHALF].reshape(P, KTILES * DHALF),
            b1.reshape(DTILES, P).T.astype(bf),
        ],
        axis=1,
    )
    w1d1 = np.ascontiguousarray(w1dk[:, :, DHALF:])

    in_maps = []
    for c in range(NCORES):
        bs = slice(c * BPC, (c + 1) * BPC)
        encT = (
            np.asarray(encoder_outputs[bs], dtype=np.float32).transpose(0, 2, 1)
            * ENC_SCALE
        )  # [BPC, E, LIN] scaled
        encC = np.ascontiguousarray(
            encT.reshape(BPC, ETILES, P, LCH, LCHW)
            .transpose(0, 3, 2, 1, 4)
            .astype(f8)
        )
        dhT = np.ascontiguousarray(d_hidden[bs].reshape(BPC, ND).T)  # [ND, BPC]
        wmisc = np.zeros((P, WMISC), dtype=bf)
        wmisc[:, 0:DH_LEN] = (
            dhT.reshape(KTILES, P, BPC).transpose(1, 0, 2).reshape(P, DH_LEN)
            .astype(bf)
        )
        wmisc[:, W2_OFF : W2_OFF + W2_LEN] = w2.reshape(DTILES, P).T.astype(bf)
        head8 = np.concatenate(
            [w1e8, encC[0, 0].reshape(P, ETILES * LCHW)], axis=1
        )
        in_maps.append(
            {
                "encC": encC,
                "head8": head8,
                "wmisc": wmisc,
                "w1d0b": w1d0b,
                "w1d1": w1d1,
            }
        )
    return in_maps


def kernel(d_hidden, encoder_outputs, W1, b1, w2):
    global LAST_RESULT
    from concourse import bass_utils

    if "nc" not in _CACHE:
        _CACHE["nc"] = _build()
    nc = _CACHE["nc"]

    in_maps = _prep_in_maps(d_hidden, encoder_outputs, W1, b1, w2)
    res = bass_utils.run_bass_kernel_spmd(
        nc,
        in_maps,
        core_ids=list(range(NCORES)),
        trace=TRACE,
        **TRACE_KWARGS,
    )
    LAST_RESULT = res
    return np.concatenate([r["out"] for r in res.results], axis=0)
